# revision 1
# baseline (speedup 1.0000x reference)
"""BinaryConv2d (3x3, SAME, NHWC) on 8 trn2 NeuronCores.

Sharding: data-parallel over batch — 2 images per core; the tiny binarized
weight tensor is replicated. Per core, the two images are packed on the two
64-partition halves of SBUF so each 3x3-tap matmul pair (K=64 contraction =
C_in) runs concurrently on disjoint row-groups of the 128x128 PE array.

Layout trick: the conv is evaluated on a flat q-grid over a zero-padded
226-wide plane, so each of the 9 taps is a pure free-dim offset
(dh*226 + dw) into the same SBUF x tile; row-crossing outputs land in 2
garbage columns per row that the host discards.
"""

import sys

for _p in ("/opt/trn_rl_repo",):
    if _p not in sys.path:
        sys.path.insert(0, _p)

import ml_dtypes
import numpy as np

BF16 = ml_dtypes.bfloat16

N_CORES = 8
IMG_PER_CORE = 2
H = W_IMG = 224
C_IN, C_OUT = 64, 128
PR, PC = 227, 226  # padded plane: 226 rows of data + 1 extra zero row
PLANE = PR * PC  # 51302
QOUT = H * PC  # 50624 q-positions per image (2 garbage cols per row)
NTAPS = 9
SLOT = 512  # q-positions per matmul (one PSUM bank of fp32)
N_SLOTS = (QOUT + SLOT - 1) // SLOT  # 99 (last slot = 448)
SLOTS_PER_CHUNK = 16
HALO = 2 * PC + 2  # 454: max tap offset
CHUNK_Q = SLOTS_PER_CHUNK * SLOT
XTILE_COLS = CHUNK_Q + HALO
STAGE_SLOTS = 8
STAGE_Q = STAGE_SLOTS * SLOT


def _chunk_plan():
    """(start_slot -> n_slots): geometric ramp so early chunks land
    just-in-time, then steady 16-slot chunks."""
    plan = {}
    s, size = 0, 2
    while s < N_SLOTS:
        n = min(size, N_SLOTS - s, SLOTS_PER_CHUNK)
        plan[s] = n
        s += n
        size *= 2
    return plan


def _stage_plan():
    """(start_slot -> n_slots): 8-slot stages, with a small split tail so
    the final out-DMA after the last matmul is tiny."""
    plan = {}
    s = 0
    while s < N_SLOTS:
        rem = N_SLOTS - s
        if rem >= STAGE_SLOTS:
            n = STAGE_SLOTS
        elif rem == 3:
            n = 2
        else:
            n = rem if rem <= 2 else rem - 1
        plan[s] = n
        s += n
    return plan

_COMPILED = None
_LAST_RES = None


def _build():
    import concourse.mybir as mybir
    import concourse.tile as tile
    from concourse import bacc

    nc = bacc.Bacc(
        "TRN2", target_bir_lowering=False, debug=False, num_devices=N_CORES
    )
    x_d = nc.dram_tensor("x", [128, PLANE], mybir.dt.bfloat16, kind="ExternalInput")
    w_d = nc.dram_tensor(
        "w", [128, NTAPS * 128], mybir.dt.bfloat16, kind="ExternalInput"
    )
    b_d = nc.dram_tensor("b", [128, 1], mybir.dt.float32, kind="ExternalInput")
    o_d = nc.dram_tensor(
        "out", [128, IMG_PER_CORE * QOUT], mybir.dt.bfloat16, kind="ExternalOutput"
    )

    ident = mybir.ActivationFunctionType.Identity

    with tile.TileContext(nc) as tc:
        with (
            tc.tile_pool(name="const", bufs=1) as cpool,
            tc.tile_pool(name="xin", bufs=4) as xpool,
            tc.tile_pool(name="stage", bufs=2) as spool,
            tc.tile_pool(name="psum", bufs=3, space="PSUM") as ppool,
        ):
            # Critical-path-first ordering on the HWDGE ring: weights, first
            # small x chunk, bias, then geometrically ramped x chunks.
            w_sb = cpool.tile([128, NTAPS * 128], mybir.dt.bfloat16, tag="w")
            nc.sync.dma_start(w_sb[:], w_d[:])
            b_sb = cpool.tile([128, 1], mybir.dt.float32, tag="b")

            # One HAM activity window (~3.4us) of dummy cold matmuls on a
            # zeroed tile, sized to finish as the first x chunk lands: the
            # PE clock-gate releases before the real stream starts, so it
            # runs at 2.4GHz from matmul 0 (results are never read).
            warm_src = cpool.tile([128, SLOT], mybir.dt.bfloat16, tag="warm")
            nc.vector.memset(warm_src[:], 0.0)
            warm_ps = ppool.tile([128, SLOT], mybir.dt.float32, tag="pswarm", bufs=1)
            N_WARM = 8
            for i in range(N_WARM):
                nc.tensor.matmul(
                    warm_ps[:, :],
                    lhsT=warm_src[:, 0:128],
                    rhs=warm_src[:, :],
                    start=(i == 0),
                    stop=(i == N_WARM - 1),
                )

            chunk_plan = _chunk_plan()
            stage_plan = _stage_plan()
            xt = None
            st_a = st_b = None
            stage_end = -1
            for s in range(N_SLOTS):
                q0 = s * SLOT
                n = min(SLOT, QOUT - q0)

                if s in chunk_plan:
                    cq0 = q0
                    ext = min(QOUT, cq0 + chunk_plan[s] * SLOT) - cq0 + HALO
                    xt = xpool.tile([128, XTILE_COLS], mybir.dt.bfloat16, tag="x")
                    nc.sync.dma_start(xt[:, :ext], x_d[:, cq0 : cq0 + ext])
                    if s == 0:
                        nc.sync.dma_start(b_sb[:], b_d[:])

                if s in stage_plan:
                    g0 = q0
                    gext = min(QOUT, g0 + stage_plan[s] * SLOT) - g0
                    stage_end = s + stage_plan[s] - 1
                    st_a = spool.tile([128, STAGE_Q], mybir.dt.bfloat16, tag="sa")
                    st_b = spool.tile([128, STAGE_Q], mybir.dt.bfloat16, tag="sb")

                psa = ppool.tile([128, SLOT], mybir.dt.float32, tag="psa")
                psb = ppool.tile([128, SLOT], mybir.dt.float32, tag="psb")

                for t in range(NTAPS):
                    dh, dw = divmod(t, 3)
                    lo = q0 - cq0 + dh * PC + dw
                    first, last = t == 0, t == NTAPS - 1
                    nc.tensor.matmul(
                        psa[:, :n],
                        lhsT=w_sb[0:64, t * 128 : (t + 1) * 128],
                        rhs=xt[0:64, lo : lo + n],
                        start=first,
                        stop=last,
                    )
                    nc.tensor.matmul(
                        psb[:, :n],
                        lhsT=w_sb[64:128, t * 128 : (t + 1) * 128],
                        rhs=xt[64:128, lo : lo + n],
                        start=first,
                        stop=last,
                    )

                so = q0 - g0
                nc.vector.tensor_scalar_add(st_a[:, so : so + n], psa[:, :n], b_sb[:])
                nc.scalar.activation(st_b[:, so : so + n], psb[:, :n], ident, bias=b_sb[:])

                if s == stage_end:
                    nc.sync.dma_start(o_d[:, g0 : g0 + gext], st_a[:, :gext])
                    nc.sync.dma_start(
                        o_d[:, QOUT + g0 : QOUT + g0 + gext], st_b[:, :gext]
                    )

    nc.compile()
    return nc


def _get_nc():
    global _COMPILED
    if _COMPILED is None:
        _COMPILED = _build()
    return _COMPILED


def kernel(x: np.ndarray, W: np.ndarray, b: np.ndarray) -> np.ndarray:
    from concourse.bass_utils import run_bass_kernel_spmd

    nc = _get_nc()

    xb = np.asarray(x, dtype=np.float32).astype(BF16)
    X = np.zeros((N_CORES, IMG_PER_CORE, C_IN, PR, PC), BF16)
    X[:, :, :, 1 : H + 1, 1 : W_IMG + 1] = xb.reshape(
        N_CORES, IMG_PER_CORE, H, W_IMG, C_IN
    ).transpose(0, 1, 4, 2, 3)
    Xf = X.reshape(N_CORES, 128, PLANE)

    Wb = np.sign(np.asarray(W, dtype=np.float32)).astype(BF16).reshape(NTAPS, C_IN, C_OUT)
    wh = np.empty((2, C_IN, NTAPS, C_OUT), BF16)
    wh[:] = Wb.transpose(1, 0, 2)[None]
    wh = np.ascontiguousarray(wh.reshape(128, NTAPS * C_OUT))

    bh = np.ascontiguousarray(np.asarray(b, dtype=np.float32).reshape(128, 1))

    in_maps = [{"x": Xf[c], "w": wh, "b": bh} for c in range(N_CORES)]
    res = run_bass_kernel_spmd(nc, in_maps, list(range(N_CORES)))
    global _LAST_RES
    _LAST_RES = res

    O = np.stack([res.results[c]["out"] for c in range(N_CORES)])
    O = O.reshape(N_CORES, C_OUT, IMG_PER_CORE, H, PC)[:, :, :, :, :W_IMG]
    y = O.transpose(0, 2, 3, 4, 1).reshape(16, H, W_IMG, C_OUT)
    return np.ascontiguousarray(y).astype(np.float32)



# revision 2
# speedup vs baseline: 1.0078x; 1.0078x over previous
"""BinaryConv2d (3x3, SAME, NHWC) on 8 trn2 NeuronCores.

Sharding: data-parallel over batch — 2 images per core; the tiny binarized
weight tensor is replicated. Per core, the two images are packed on the two
64-partition halves of SBUF so each 3x3-tap matmul pair (K=64 contraction =
C_in) runs concurrently on disjoint row-groups of the 128x128 PE array.

Each matmul covers exactly 2 output rows (N = 2*224 = 448) via a 2D rhs
access pattern over the zero-padded 226-wide plane, so the output stream is
dense — no garbage columns and 112 uniform slots per image.
"""

import sys

for _p in ("/opt/trn_rl_repo",):
    if _p not in sys.path:
        sys.path.insert(0, _p)

import ml_dtypes
import numpy as np

BF16 = ml_dtypes.bfloat16

N_CORES = 8
IMG_PER_CORE = 2
H = W_IMG = 224
C_IN, C_OUT = 64, 128
PR = PC = 226  # padded plane: 224 data rows/cols + 1 zero ring
PLANE = PR * PC  # 51076
NSLOT = 2 * W_IMG  # 448 outputs per matmul = 2 dense image rows
N_SLOTS = 112  # 112 * 448 = 50176 = 224*224, exact
NTAPS = 9
QOUT = H * W_IMG  # 50176 dense outputs per image

# slots per x-input chunk: small first chunk lands just-in-time, then steady
CHUNKS = [1, 2, 4, 8, 16, 16, 16, 16, 16, 17]
XTILE_ROWS = 2 * max(CHUNKS) + 2  # 36 padded rows
XTILE_COLS = XTILE_ROWS * PC  # 8136
# slots per output stage: small tail stages so the final out-DMA is tiny
STAGES = [8] * 13 + [4, 2, 1, 1]
STAGE_Q = 8 * NSLOT  # 3584
N_WARM = 6

_COMPILED = None
_LAST_RES = None


def _build():
    import concourse.mybir as mybir
    import concourse.tile as tile
    from concourse import bacc

    nc = bacc.Bacc(
        "TRN2", target_bir_lowering=False, debug=False, num_devices=N_CORES
    )
    x_d = nc.dram_tensor("x", [128, PLANE], mybir.dt.bfloat16, kind="ExternalInput")
    w_d = nc.dram_tensor(
        "w", [128, NTAPS * 128], mybir.dt.bfloat16, kind="ExternalInput"
    )
    b_d = nc.dram_tensor("b", [128, 1], mybir.dt.float32, kind="ExternalInput")
    o_d = nc.dram_tensor(
        "out", [128, IMG_PER_CORE * QOUT], mybir.dt.bfloat16, kind="ExternalOutput"
    )

    ident = mybir.ActivationFunctionType.Identity

    chunk_plan = {}
    s = 0
    for n in CHUNKS:
        chunk_plan[s] = n
        s += n
    assert s == N_SLOTS
    stage_plan = {}
    s = 0
    for n in STAGES:
        stage_plan[s] = n
        s += n
    assert s == N_SLOTS

    with tile.TileContext(nc) as tc:
        with (
            tc.tile_pool(name="const", bufs=1) as cpool,
            tc.tile_pool(name="xin", bufs=4) as xpool,
            tc.tile_pool(name="stage", bufs=3) as spool,
            tc.tile_pool(name="psum", bufs=3, space="PSUM") as ppool,
        ):
            # Critical-path-first ordering on the HWDGE ring: weights, then
            # the 1-slot first x chunk, bias, then geometrically ramped chunks.
            w_sb = cpool.tile([128, NTAPS * 128], mybir.dt.bfloat16, tag="w")
            nc.sync.dma_start(w_sb[:], w_d[:])
            b_sb = cpool.tile([128, 1], mybir.dt.float32, tag="b")

            # One HAM activity window (~3.4us) of dummy cold matmuls on a
            # zeroed tile, sized to finish as the first x chunk lands: the
            # PE clock-gate releases before the real stream starts, so it
            # runs at 2.4GHz early in the real stream (results never read).
            warm_src = cpool.tile([128, NSLOT], mybir.dt.bfloat16, tag="warm")
            nc.vector.memset(warm_src[:], 0.0)
            warm_ps = ppool.tile([128, 512], mybir.dt.float32, tag="pswarm", bufs=1)
            for i in range(N_WARM):
                nc.tensor.matmul(
                    warm_ps[:, :NSLOT],
                    lhsT=warm_src[:, 0:128],
                    rhs=warm_src[:, :],
                    start=(i == 0),
                    stop=(i == N_WARM - 1),
                )

            xv = None
            ca = 0
            st_a = st_b = None
            g0 = 0
            gext = 0
            stage_end = -1
            for s in range(N_SLOTS):
                if s in chunk_plan:
                    ca = s
                    n_c = chunk_plan[s]
                    ext = (2 * n_c + 2) * PC
                    xt = xpool.tile([128, XTILE_COLS], mybir.dt.bfloat16, tag="x")
                    nc.sync.dma_start(xt[:, :ext], x_d[:, 2 * ca * PC : 2 * ca * PC + ext])
                    if s == 0:
                        nc.sync.dma_start(b_sb[:], b_d[:])
                    xv = xt[:, :ext].rearrange("p (r w) -> p r w", w=PC)

                if s in stage_plan:
                    g0 = s * NSLOT
                    gext = stage_plan[s] * NSLOT
                    stage_end = s + stage_plan[s] - 1
                    st_a = spool.tile([128, STAGE_Q], mybir.dt.bfloat16, tag="sa")
                    st_b = spool.tile([128, STAGE_Q], mybir.dt.bfloat16, tag="sb")

                psa = ppool.tile([128, 512], mybir.dt.float32, tag="psa")
                psb = ppool.tile([128, 512], mybir.dt.float32, tag="psb")

                for t in range(NTAPS):
                    dh, dw = divmod(t, 3)
                    r0 = 2 * (s - ca) + dh
                    first, last = t == 0, t == NTAPS - 1
                    nc.tensor.matmul(
                        psa[:, :NSLOT],
                        lhsT=w_sb[0:64, t * 128 : (t + 1) * 128],
                        rhs=xv[0:64, r0 : r0 + 2, dw : dw + W_IMG],
                        start=first,
                        stop=last,
                    )
                    nc.tensor.matmul(
                        psb[:, :NSLOT],
                        lhsT=w_sb[64:128, t * 128 : (t + 1) * 128],
                        rhs=xv[64:128, r0 : r0 + 2, dw : dw + W_IMG],
                        start=first,
                        stop=last,
                    )

                so = s * NSLOT - g0
                nc.vector.tensor_scalar_add(
                    st_a[:, so : so + NSLOT], psa[:, :NSLOT], b_sb[:]
                )
                nc.scalar.activation(
                    st_b[:, so : so + NSLOT], psb[:, :NSLOT], ident, bias=b_sb[:]
                )

                if s == stage_end:
                    nc.sync.dma_start(o_d[:, g0 : g0 + gext], st_a[:, :gext])
                    nc.scalar.dma_start(
                        o_d[:, QOUT + g0 : QOUT + g0 + gext], st_b[:, :gext]
                    )

    nc.compile()
    return nc


def _get_nc():
    global _COMPILED
    if _COMPILED is None:
        _COMPILED = _build()
    return _COMPILED


def kernel(x: np.ndarray, W: np.ndarray, b: np.ndarray) -> np.ndarray:
    from concourse.bass_utils import run_bass_kernel_spmd

    nc = _get_nc()

    xb = np.asarray(x, dtype=np.float32).astype(BF16)
    X = np.zeros((N_CORES, IMG_PER_CORE, C_IN, PR, PC), BF16)
    X[:, :, :, 1 : H + 1, 1 : W_IMG + 1] = xb.reshape(
        N_CORES, IMG_PER_CORE, H, W_IMG, C_IN
    ).transpose(0, 1, 4, 2, 3)
    Xf = X.reshape(N_CORES, 128, PLANE)

    Wb = np.sign(np.asarray(W, dtype=np.float32)).astype(BF16).reshape(NTAPS, C_IN, C_OUT)
    wh = np.empty((2, C_IN, NTAPS, C_OUT), BF16)
    wh[:] = Wb.transpose(1, 0, 2)[None]
    wh = np.ascontiguousarray(wh.reshape(128, NTAPS * C_OUT))

    bh = np.ascontiguousarray(np.asarray(b, dtype=np.float32).reshape(128, 1))

    in_maps = [{"x": Xf[c], "w": wh, "b": bh} for c in range(N_CORES)]
    res = run_bass_kernel_spmd(nc, in_maps, list(range(N_CORES)))
    global _LAST_RES
    _LAST_RES = res

    O = np.stack([res.results[c]["out"] for c in range(N_CORES)])
    O = O.reshape(N_CORES, C_OUT, IMG_PER_CORE, H, W_IMG)
    y = O.transpose(0, 2, 3, 4, 1).reshape(16, H, W_IMG, C_OUT)
    return np.ascontiguousarray(y).astype(np.float32)


# revision 5
# speedup vs baseline: 1.0082x; 1.0004x over previous
"""BinaryConv2d (3x3, SAME, NHWC) on 8 trn2 NeuronCores.

Sharding: data-parallel over batch — 2 images per core; the tiny binarized
weight tensor is replicated. Per core, the two images are packed on the two
64-partition halves of SBUF so each 3x3-tap matmul pair (K=64 contraction =
C_in) runs concurrently on disjoint row-groups of the 128x128 PE array.

Each matmul covers exactly 2 output rows (N = 2*224 = 448) via a 2D rhs
access pattern over the zero-padded 226-wide plane, so the output stream is
dense — no garbage columns and 112 uniform slots per image.
"""

import sys

for _p in ("/opt/trn_rl_repo",):
    if _p not in sys.path:
        sys.path.insert(0, _p)

import ml_dtypes
import numpy as np

BF16 = ml_dtypes.bfloat16

N_CORES = 8
IMG_PER_CORE = 2
H = W_IMG = 224
C_IN, C_OUT = 64, 128
PR = PC = 226  # padded plane: 224 data rows/cols + 1 zero ring
PLANE = PR * PC  # 51076
NSLOT = 2 * W_IMG  # 448 outputs per matmul = 2 dense image rows
N_SLOTS = 112  # 112 * 448 = 50176 = 224*224, exact
NTAPS = 9
QOUT = H * W_IMG  # 50176 dense outputs per image

# slots per x-input chunk: small first chunk lands just-in-time, then steady
CHUNKS = [1, 2, 4, 8, 16, 16, 16, 16, 16, 17]
XTILE_ROWS = 2 * max(CHUNKS) + 2  # 36 padded rows
XTILE_COLS = XTILE_ROWS * PC  # 8136
# slots per output stage: small tail stages so the final out-DMA is tiny
STAGES = [8] * 13 + [4, 2, 1, 1]
STAGE_Q = 8 * NSLOT  # 3584
N_WARM = 5

_COMPILED = None
_LAST_RES = None


def _build():
    import concourse.mybir as mybir
    import concourse.tile as tile
    from concourse import bacc

    nc = bacc.Bacc(
        "TRN2", target_bir_lowering=False, debug=False, num_devices=N_CORES
    )
    x_d = nc.dram_tensor("x", [128, PLANE], mybir.dt.bfloat16, kind="ExternalInput")
    w_d = nc.dram_tensor(
        "w", [128, NTAPS * 128], mybir.dt.bfloat16, kind="ExternalInput"
    )
    b_d = nc.dram_tensor("b", [128, 1], mybir.dt.float32, kind="ExternalInput")
    o_d = nc.dram_tensor(
        "out", [128, IMG_PER_CORE * QOUT], mybir.dt.bfloat16, kind="ExternalOutput"
    )

    ident = mybir.ActivationFunctionType.Identity

    chunk_plan = {}
    s = 0
    for n in CHUNKS:
        chunk_plan[s] = n
        s += n
    assert s == N_SLOTS
    stage_plan = {}
    s = 0
    for n in STAGES:
        stage_plan[s] = n
        s += n
    assert s == N_SLOTS

    with tile.TileContext(nc) as tc:
        with (
            tc.tile_pool(name="const", bufs=1) as cpool,
            tc.tile_pool(name="xin", bufs=4) as xpool,
            tc.tile_pool(name="stage", bufs=3) as spool,
            tc.tile_pool(name="psum", bufs=3, space="PSUM") as ppool,
        ):
            # Need-ordered fine-grained first transfers on the HWDGE ring:
            # each piece lands just before its first cold matmul consumes it
            # (the first DMA packet on each engine pays a ~0.5us startup).
            w_sb = cpool.tile([128, NTAPS * 128], mybir.dt.bfloat16, tag="w")
            b_sb = cpool.tile([128, 1], mybir.dt.float32, tag="b")
            nc.sync.dma_start(w_sb[:, 0:128], w_d[:, 0:128])  # tap 0

            # One HAM activity window (~3.4us) of dummy cold matmuls on a
            # zeroed tile, sized to finish as the first x chunk lands: the
            # PE clock-gate releases before the real stream starts, so it
            # runs at 2.4GHz early in the real stream (results never read).
            warm_src = cpool.tile([128, NSLOT], mybir.dt.bfloat16, tag="warm")
            nc.vector.memset(warm_src[:], 0.0)
            warm_ps = ppool.tile([128, 512], mybir.dt.float32, tag="pswarm", bufs=1)
            for i in range(N_WARM):
                nc.tensor.matmul(
                    warm_ps[:, :NSLOT],
                    lhsT=warm_src[:, 0:128],
                    rhs=warm_src[:, :],
                    start=(i == 0),
                    stop=(i == N_WARM - 1),
                )

            xv = None
            ca = 0
            st_a = st_b = None
            g0 = 0
            gext = 0
            stage_end = -1
            for s in range(N_SLOTS):
                if s in chunk_plan:
                    ca = s
                    n_c = chunk_plan[s]
                    ext = (2 * n_c + 2) * PC
                    xt = xpool.tile([128, XTILE_COLS], mybir.dt.bfloat16, tag="x")
                    if s == 0:
                        # interleave x rows with weight taps in need-order
                        nc.sync.dma_start(xt[:, : 2 * PC], x_d[:, : 2 * PC])
                        nc.sync.dma_start(w_sb[:, 128:384], w_d[:, 128:384])
                        nc.sync.dma_start(
                            xt[:, 2 * PC : 4 * PC], x_d[:, 2 * PC : 4 * PC]
                        )
                        nc.sync.dma_start(b_sb[:], b_d[:])
                        nc.sync.dma_start(w_sb[:, 384:512], w_d[:, 384:512])
                        nc.sync.dma_start(w_sb[:, 512:1152], w_d[:, 512:1152])
                    else:
                        nc.sync.dma_start(
                            xt[:, :ext], x_d[:, 2 * ca * PC : 2 * ca * PC + ext]
                        )
                    xv = xt[:, :ext].rearrange("p (r w) -> p r w", w=PC)

                if s in stage_plan:
                    g0 = s * NSLOT
                    gext = stage_plan[s] * NSLOT
                    stage_end = s + stage_plan[s] - 1
                    st_a = spool.tile([128, STAGE_Q], mybir.dt.bfloat16, tag="sa")
                    st_b = spool.tile([128, STAGE_Q], mybir.dt.bfloat16, tag="sb")

                psa = ppool.tile([128, 512], mybir.dt.float32, tag="psa")
                psb = ppool.tile([128, 512], mybir.dt.float32, tag="psb")

                for t in range(NTAPS):
                    dh, dw = divmod(t, 3)
                    r0 = 2 * (s - ca) + dh
                    first, last = t == 0, t == NTAPS - 1
                    nc.tensor.matmul(
                        psa[:, :NSLOT],
                        lhsT=w_sb[0:64, t * 128 : (t + 1) * 128],
                        rhs=xv[0:64, r0 : r0 + 2, dw : dw + W_IMG],
                        start=first,
                        stop=last,
                    )
                    nc.tensor.matmul(
                        psb[:, :NSLOT],
                        lhsT=w_sb[64:128, t * 128 : (t + 1) * 128],
                        rhs=xv[64:128, r0 : r0 + 2, dw : dw + W_IMG],
                        start=first,
                        stop=last,
                    )

                so = s * NSLOT - g0
                nc.vector.tensor_scalar_add(
                    st_a[:, so : so + NSLOT], psa[:, :NSLOT], b_sb[:]
                )
                nc.scalar.activation(
                    st_b[:, so : so + NSLOT], psb[:, :NSLOT], ident, bias=b_sb[:]
                )

                if s == stage_end:
                    nc.sync.dma_start(o_d[:, g0 : g0 + gext], st_a[:, :gext])
                    nc.scalar.dma_start(
                        o_d[:, QOUT + g0 : QOUT + g0 + gext], st_b[:, :gext]
                    )

    nc.compile()
    return nc


def _get_nc():
    global _COMPILED
    if _COMPILED is None:
        _COMPILED = _build()
    return _COMPILED


def kernel(x: np.ndarray, W: np.ndarray, b: np.ndarray) -> np.ndarray:
    from concourse.bass_utils import run_bass_kernel_spmd

    nc = _get_nc()

    xb = np.asarray(x, dtype=np.float32).astype(BF16)
    X = np.zeros((N_CORES, IMG_PER_CORE, C_IN, PR, PC), BF16)
    X[:, :, :, 1 : H + 1, 1 : W_IMG + 1] = xb.reshape(
        N_CORES, IMG_PER_CORE, H, W_IMG, C_IN
    ).transpose(0, 1, 4, 2, 3)
    Xf = X.reshape(N_CORES, 128, PLANE)

    Wb = np.sign(np.asarray(W, dtype=np.float32)).astype(BF16).reshape(NTAPS, C_IN, C_OUT)
    wh = np.empty((2, C_IN, NTAPS, C_OUT), BF16)
    wh[:] = Wb.transpose(1, 0, 2)[None]
    wh = np.ascontiguousarray(wh.reshape(128, NTAPS * C_OUT))

    bh = np.ascontiguousarray(np.asarray(b, dtype=np.float32).reshape(128, 1))

    in_maps = [{"x": Xf[c], "w": wh, "b": bh} for c in range(N_CORES)]
    res = run_bass_kernel_spmd(nc, in_maps, list(range(N_CORES)))
    global _LAST_RES
    _LAST_RES = res

    O = np.stack([res.results[c]["out"] for c in range(N_CORES)])
    O = O.reshape(N_CORES, C_OUT, IMG_PER_CORE, H, W_IMG)
    y = O.transpose(0, 2, 3, 4, 1).reshape(16, H, W_IMG, C_OUT)
    return np.ascontiguousarray(y).astype(np.float32)


# revision 11
# speedup vs baseline: 1.0096x; 1.0014x over previous
"""BinaryConv2d (3x3, SAME, NHWC) on 8 trn2 NeuronCores.

Sharding: data-parallel over batch — 2 images per core; the tiny binarized
weight tensor is replicated. Per core, the two images are packed on the two
64-partition halves of SBUF so each 3x3-tap matmul pair (K=64 contraction =
C_in) runs concurrently on disjoint row-groups of the 128x128 PE array.

Each matmul covers exactly 2 output rows (N = 2*224 = 448) via a 2D rhs
access pattern over the zero-padded 226-wide plane, so the output stream is
dense — no garbage columns and 112 uniform slots per image.
"""

import sys

for _p in ("/opt/trn_rl_repo",):
    if _p not in sys.path:
        sys.path.insert(0, _p)

import ml_dtypes
import numpy as np

BF16 = ml_dtypes.bfloat16

N_CORES = 8
IMG_PER_CORE = 2
H = W_IMG = 224
C_IN, C_OUT = 64, 128
PR = PC = 226  # padded plane: 224 data rows/cols + 1 zero ring
PLANE = PR * PC  # 51076
NSLOT = 2 * W_IMG  # 448 outputs per matmul = 2 dense image rows
N_SLOTS = 112  # 112 * 448 = 50176 = 224*224, exact
NTAPS = 9
QOUT = H * W_IMG  # 50176 dense outputs per image

# slots per x-input chunk: small first chunk lands just-in-time, then steady
CHUNKS = [1, 2, 4, 8, 16, 16, 16, 16, 16, 17]
XTILE_ROWS = 2 * max(CHUNKS) + 2  # 36 padded rows
XTILE_COLS = XTILE_ROWS * PC  # 8136
# slots per output stage: small tail stages so the final out-DMA is tiny
STAGES = [8] * 13 + [4, 2, 1, 1]
STAGE_Q = 8 * NSLOT  # 3584
N_WARM = 7
HOT_COLS = 3 * 128 + 4 * PC  # w taps 0-2, then x_pad rows 0-3: 1288

_COMPILED = None
_LAST_RES = None


def _build():
    import concourse.mybir as mybir
    import concourse.tile as tile
    from concourse import bacc

    nc = bacc.Bacc(
        "TRN2", target_bir_lowering=False, debug=False, num_devices=N_CORES
    )
    x_d = nc.dram_tensor("x", [128, PLANE], mybir.dt.bfloat16, kind="ExternalInput")
    w_d = nc.dram_tensor(
        "w", [128, NTAPS * 128], mybir.dt.bfloat16, kind="ExternalInput"
    )
    hot_d = nc.dram_tensor(
        "hot", [128, HOT_COLS], mybir.dt.bfloat16, kind="ExternalInput"
    )
    b_d = nc.dram_tensor("b", [128, 1], mybir.dt.float32, kind="ExternalInput")
    o_d = nc.dram_tensor(
        "out", [128, IMG_PER_CORE * QOUT], mybir.dt.bfloat16, kind="ExternalOutput"
    )

    ident = mybir.ActivationFunctionType.Identity

    chunk_plan = {}
    s = 0
    for n in CHUNKS:
        chunk_plan[s] = n
        s += n
    assert s == N_SLOTS
    stage_plan = {}
    s = 0
    for n in STAGES:
        stage_plan[s] = n
        s += n
    assert s == N_SLOTS

    with tile.TileContext(nc) as tc:
        with (
            tc.tile_pool(name="const", bufs=1) as cpool,
            tc.tile_pool(name="xin", bufs=4) as xpool,
            tc.tile_pool(name="stage", bufs=3) as spool,
            tc.tile_pool(name="psum", bufs=3, space="PSUM") as ppool,
        ):
            # Each DMA has a ~2.4us fixed issue->semaphore latency, so the
            # whole slot-0 working set (w taps 0-2 + x_pad rows 0-3) ships as
            # ONE early "hot" transfer; taps 0-2 are served from this
            # resident tile for every slot. Taps 3-8 follow in a second DMA
            # that lands before the cold stream reaches tap 3.
            ht = cpool.tile([128, HOT_COLS], mybir.dt.bfloat16, tag="hot")
            nc.sync.dma_start(ht[:], hot_d[:])
            w_sb = cpool.tile([128, NTAPS * 128], mybir.dt.bfloat16, tag="w")
            nc.sync.dma_start(w_sb[:, 384:1152], w_d[:, 384:1152])
            b_sb = cpool.tile([128, 1], mybir.dt.float32, tag="b")
            xv0 = ht[:, 384 : 384 + 4 * PC].rearrange("p (r w) -> p r w", w=PC)

            # One HAM activity window (~3.4us) of dummy cold matmuls on a
            # zeroed tile, sized to finish as the first x chunk lands: the
            # PE clock-gate releases before the real stream starts, so it
            # runs at 2.4GHz early in the real stream (results never read).
            warm_src = cpool.tile([128, NSLOT], mybir.dt.bfloat16, tag="warm")
            nc.vector.memset(warm_src[:], 0.0)
            warm_ps = ppool.tile([128, 512], mybir.dt.float32, tag="pswarm", bufs=1)
            for i in range(N_WARM):
                nc.tensor.matmul(
                    warm_ps[:, :NSLOT],
                    lhsT=warm_src[:, 0:128],
                    rhs=warm_src[:, :],
                    start=(i == 0),
                    stop=(i == N_WARM - 1),
                )

            xv = None
            ca = 0
            st_a = st_b = None
            g0 = 0
            gext = 0
            stage_end = -1
            for s in range(N_SLOTS):
                if s in chunk_plan:
                    ca = s
                    if s == 0:
                        xv = xv0  # slot 0 reads the resident hot tile
                        nc.sync.dma_start(b_sb[:], b_d[:])
                    else:
                        n_c = chunk_plan[s]
                        ext = (2 * n_c + 2) * PC
                        xt = xpool.tile([128, XTILE_COLS], mybir.dt.bfloat16, tag="x")
                        nc.sync.dma_start(
                            xt[:, :ext], x_d[:, 2 * ca * PC : 2 * ca * PC + ext]
                        )
                        xv = xt[:, :ext].rearrange("p (r w) -> p r w", w=PC)

                if s in stage_plan:
                    g0 = s * NSLOT
                    gext = stage_plan[s] * NSLOT
                    stage_end = s + stage_plan[s] - 1
                    st_a = spool.tile([128, STAGE_Q], mybir.dt.bfloat16, tag="sa")
                    st_b = spool.tile([128, STAGE_Q], mybir.dt.bfloat16, tag="sb")

                psa = ppool.tile([128, 512], mybir.dt.float32, tag="psa")
                psb = ppool.tile([128, 512], mybir.dt.float32, tag="psb")

                for t in range(NTAPS):
                    dh, dw = divmod(t, 3)
                    r0 = 2 * (s - ca) + dh
                    first, last = t == 0, t == NTAPS - 1
                    wt = ht if t < 3 else w_sb
                    nc.tensor.matmul(
                        psa[:, :NSLOT],
                        lhsT=wt[0:64, t * 128 : (t + 1) * 128],
                        rhs=xv[0:64, r0 : r0 + 2, dw : dw + W_IMG],
                        start=first,
                        stop=last,
                    )
                    nc.tensor.matmul(
                        psb[:, :NSLOT],
                        lhsT=wt[64:128, t * 128 : (t + 1) * 128],
                        rhs=xv[64:128, r0 : r0 + 2, dw : dw + W_IMG],
                        start=first,
                        stop=last,
                    )

                so = s * NSLOT - g0
                nc.vector.tensor_scalar_add(
                    st_a[:, so : so + NSLOT], psa[:, :NSLOT], b_sb[:]
                )
                nc.scalar.activation(
                    st_b[:, so : so + NSLOT], psb[:, :NSLOT], ident, bias=b_sb[:]
                )

                if s == stage_end:
                    nc.sync.dma_start(o_d[:, g0 : g0 + gext], st_a[:, :gext])
                    nc.scalar.dma_start(
                        o_d[:, QOUT + g0 : QOUT + g0 + gext], st_b[:, :gext]
                    )

    nc.compile()
    return nc


def _get_nc():
    global _COMPILED
    if _COMPILED is None:
        _COMPILED = _build()
    return _COMPILED


def kernel(x: np.ndarray, W: np.ndarray, b: np.ndarray) -> np.ndarray:
    from concourse.bass_utils import run_bass_kernel_spmd

    nc = _get_nc()

    xb = np.asarray(x, dtype=np.float32).astype(BF16)
    X = np.zeros((N_CORES, IMG_PER_CORE, C_IN, PR, PC), BF16)
    X[:, :, :, 1 : H + 1, 1 : W_IMG + 1] = xb.reshape(
        N_CORES, IMG_PER_CORE, H, W_IMG, C_IN
    ).transpose(0, 1, 4, 2, 3)
    Xf = X.reshape(N_CORES, 128, PLANE)

    Wb = np.sign(np.asarray(W, dtype=np.float32)).astype(BF16).reshape(NTAPS, C_IN, C_OUT)
    wh = np.empty((2, C_IN, NTAPS, C_OUT), BF16)
    wh[:] = Wb.transpose(1, 0, 2)[None]
    wh = np.ascontiguousarray(wh.reshape(128, NTAPS * C_OUT))

    bh = np.ascontiguousarray(np.asarray(b, dtype=np.float32).reshape(128, 1))

    hot = [
        np.ascontiguousarray(
            np.concatenate([wh[:, : 3 * 128], Xf[c][:, : 4 * PC]], axis=1)
        )
        for c in range(N_CORES)
    ]

    in_maps = [
        {"x": Xf[c], "w": wh, "b": bh, "hot": hot[c]} for c in range(N_CORES)
    ]
    res = run_bass_kernel_spmd(nc, in_maps, list(range(N_CORES)))
    global _LAST_RES
    _LAST_RES = res

    O = np.stack([res.results[c]["out"] for c in range(N_CORES)])
    O = O.reshape(N_CORES, C_OUT, IMG_PER_CORE, H, W_IMG)
    y = O.transpose(0, 2, 3, 4, 1).reshape(16, H, W_IMG, C_OUT)
    return np.ascontiguousarray(y).astype(np.float32)


# revision 14
# speedup vs baseline: 1.0133x; 1.0037x over previous
"""BinaryConv2d (3x3, SAME, NHWC) on 8 trn2 NeuronCores.

Sharding: data-parallel over batch — 2 images per core; the tiny binarized
weight tensor is replicated. Per core, the two images are packed on the two
64-partition halves of SBUF so each 3x3-tap matmul pair (K=64 contraction =
C_in) runs concurrently on disjoint row-groups of the 128x128 PE array.

Each matmul covers exactly 2 output rows (N = 2*224 = 448) via a 2D rhs
access pattern over the zero-padded 226-wide plane, so the output stream is
dense — no garbage columns and 112 uniform slots per image.
"""

import sys

for _p in ("/opt/trn_rl_repo",):
    if _p not in sys.path:
        sys.path.insert(0, _p)

import ml_dtypes
import numpy as np

BF16 = ml_dtypes.bfloat16

N_CORES = 8
IMG_PER_CORE = 2
H = W_IMG = 224
C_IN, C_OUT = 64, 128
PR = PC = 226  # padded plane: 224 data rows/cols + 1 zero ring
PLANE = PR * PC  # 51076
NSLOT = 2 * W_IMG  # 448 outputs per matmul = 2 dense image rows
N_SLOTS = 112  # 112 * 448 = 50176 = 224*224, exact
NTAPS = 9
QOUT = H * W_IMG  # 50176 dense outputs per image

# slots per x-input chunk: first 2 slots ride in the hot transfer, then a
# supply-matched ramp (head DMA bandwidth ramps ~100->450 GB/s over ~10us)
CHUNKS = [2, 2, 4, 8, 16, 16, 16, 16, 16, 16]
XTILE_ROWS = 2 * max(CHUNKS) + 2  # 34 padded rows
XTILE_COLS = XTILE_ROWS * PC  # 7684
# slots per output stage: small tail stages so the final out-DMA is tiny
STAGES = [8] * 13 + [4, 2, 1, 1]
STAGE_Q = 8 * NSLOT  # 3584
N_WARM = 9
HOT_ROWS = 6  # x_pad rows 0-5: slots 0-1
HOT_COLS = 3 * 128 + HOT_ROWS * PC  # w taps 0-2, then x rows: 1740

_COMPILED = None
_LAST_RES = None


def _build():
    import concourse.mybir as mybir
    import concourse.tile as tile
    from concourse import bacc

    nc = bacc.Bacc(
        "TRN2", target_bir_lowering=False, debug=False, num_devices=N_CORES
    )
    x_d = nc.dram_tensor("x", [128, PLANE], mybir.dt.bfloat16, kind="ExternalInput")
    w_d = nc.dram_tensor(
        "w", [128, NTAPS * 128], mybir.dt.bfloat16, kind="ExternalInput"
    )
    hot_d = nc.dram_tensor(
        "hot", [128, HOT_COLS], mybir.dt.bfloat16, kind="ExternalInput"
    )
    b_d = nc.dram_tensor("b", [128, 1], mybir.dt.float32, kind="ExternalInput")
    o_d = nc.dram_tensor(
        "out", [128, IMG_PER_CORE * QOUT], mybir.dt.bfloat16, kind="ExternalOutput"
    )

    ident = mybir.ActivationFunctionType.Identity

    chunk_plan = {}
    s = 0
    for n in CHUNKS:
        chunk_plan[s] = n
        s += n
    assert s == N_SLOTS
    stage_plan = {}
    s = 0
    for n in STAGES:
        stage_plan[s] = n
        s += n
    assert s == N_SLOTS

    with tile.TileContext(nc) as tc:
        with (
            tc.tile_pool(name="const", bufs=1) as cpool,
            tc.tile_pool(name="xin", bufs=4) as xpool,
            tc.tile_pool(name="stage", bufs=3) as spool,
            tc.tile_pool(name="psum", bufs=3, space="PSUM") as ppool,
        ):
            # Each DMA has a ~2.4us fixed issue->semaphore latency, so the
            # whole slot-0 working set (w taps 0-2 + x_pad rows 0-3) ships as
            # ONE early "hot" transfer; taps 0-2 are served from this
            # resident tile for every slot. Taps 3-8 follow in a second DMA
            # that lands before the cold stream reaches tap 3.
            ht = cpool.tile([128, HOT_COLS], mybir.dt.bfloat16, tag="hot")
            nc.sync.dma_start(ht[:], hot_d[:])
            w_sb = cpool.tile([128, NTAPS * 128], mybir.dt.bfloat16, tag="w")
            nc.sync.dma_start(w_sb[:, 384:1152], w_d[:, 384:1152])
            b_sb = cpool.tile([128, 1], mybir.dt.float32, tag="b")
            xv0 = ht[:, 384 : 384 + HOT_ROWS * PC].rearrange("p (r w) -> p r w", w=PC)

            # One HAM activity window (~3.4us) of dummy cold matmuls on a
            # zeroed tile, sized to finish as the first x chunk lands: the
            # PE clock-gate releases before the real stream starts, so it
            # runs at 2.4GHz early in the real stream (results never read).
            warm_src = cpool.tile([128, NSLOT], mybir.dt.bfloat16, tag="warm")
            nc.vector.memset(warm_src[:], 0.0)
            warm_ps = ppool.tile([128, 512], mybir.dt.float32, tag="pswarm", bufs=1)
            for i in range(N_WARM):
                nc.tensor.matmul(
                    warm_ps[:, :NSLOT],
                    lhsT=warm_src[:, 0:128],
                    rhs=warm_src[:, :],
                    start=(i == 0),
                    stop=(i == N_WARM - 1),
                )

            xv = None
            ca = 0
            st_a = st_b = None
            g0 = 0
            gext = 0
            stage_end = -1
            for s in range(N_SLOTS):
                if s in chunk_plan:
                    ca = s
                    if s == 0:
                        xv = xv0  # slot 0 reads the resident hot tile
                        nc.sync.dma_start(b_sb[:], b_d[:])
                    else:
                        n_c = chunk_plan[s]
                        ext = (2 * n_c + 2) * PC
                        xt = xpool.tile([128, XTILE_COLS], mybir.dt.bfloat16, tag="x")
                        nc.sync.dma_start(
                            xt[:, :ext], x_d[:, 2 * ca * PC : 2 * ca * PC + ext]
                        )
                        xv = xt[:, :ext].rearrange("p (r w) -> p r w", w=PC)

                if s in stage_plan:
                    g0 = s * NSLOT
                    gext = stage_plan[s] * NSLOT
                    stage_end = s + stage_plan[s] - 1
                    st_a = spool.tile([128, STAGE_Q], mybir.dt.bfloat16, tag="sa")
                    st_b = spool.tile([128, STAGE_Q], mybir.dt.bfloat16, tag="sb")

                psa = ppool.tile([128, 512], mybir.dt.float32, tag="psa")
                psb = ppool.tile([128, 512], mybir.dt.float32, tag="psb")

                for t in range(NTAPS):
                    dh, dw = divmod(t, 3)
                    r0 = 2 * (s - ca) + dh
                    first, last = t == 0, t == NTAPS - 1
                    wt = ht if t < 3 else w_sb
                    nc.tensor.matmul(
                        psa[:, :NSLOT],
                        lhsT=wt[0:64, t * 128 : (t + 1) * 128],
                        rhs=xv[0:64, r0 : r0 + 2, dw : dw + W_IMG],
                        start=first,
                        stop=last,
                    )
                    nc.tensor.matmul(
                        psb[:, :NSLOT],
                        lhsT=wt[64:128, t * 128 : (t + 1) * 128],
                        rhs=xv[64:128, r0 : r0 + 2, dw : dw + W_IMG],
                        start=first,
                        stop=last,
                    )

                so = s * NSLOT - g0
                nc.vector.tensor_scalar_add(
                    st_a[:, so : so + NSLOT], psa[:, :NSLOT], b_sb[:]
                )
                nc.scalar.activation(
                    st_b[:, so : so + NSLOT], psb[:, :NSLOT], ident, bias=b_sb[:]
                )

                if s == stage_end:
                    nc.sync.dma_start(o_d[:, g0 : g0 + gext], st_a[:, :gext])
                    nc.scalar.dma_start(
                        o_d[:, QOUT + g0 : QOUT + g0 + gext], st_b[:, :gext]
                    )

    nc.compile()
    return nc


def _get_nc():
    global _COMPILED
    if _COMPILED is None:
        _COMPILED = _build()
    return _COMPILED


def kernel(x: np.ndarray, W: np.ndarray, b: np.ndarray) -> np.ndarray:
    from concourse.bass_utils import run_bass_kernel_spmd

    nc = _get_nc()

    xb = np.asarray(x, dtype=np.float32).astype(BF16)
    X = np.zeros((N_CORES, IMG_PER_CORE, C_IN, PR, PC), BF16)
    X[:, :, :, 1 : H + 1, 1 : W_IMG + 1] = xb.reshape(
        N_CORES, IMG_PER_CORE, H, W_IMG, C_IN
    ).transpose(0, 1, 4, 2, 3)
    Xf = X.reshape(N_CORES, 128, PLANE)

    Wb = np.sign(np.asarray(W, dtype=np.float32)).astype(BF16).reshape(NTAPS, C_IN, C_OUT)
    wh = np.empty((2, C_IN, NTAPS, C_OUT), BF16)
    wh[:] = Wb.transpose(1, 0, 2)[None]
    wh = np.ascontiguousarray(wh.reshape(128, NTAPS * C_OUT))

    bh = np.ascontiguousarray(np.asarray(b, dtype=np.float32).reshape(128, 1))

    hot = [
        np.ascontiguousarray(
            np.concatenate([wh[:, : 3 * 128], Xf[c][:, : HOT_ROWS * PC]], axis=1)
        )
        for c in range(N_CORES)
    ]

    in_maps = [
        {"x": Xf[c], "w": wh, "b": bh, "hot": hot[c]} for c in range(N_CORES)
    ]
    res = run_bass_kernel_spmd(nc, in_maps, list(range(N_CORES)))
    global _LAST_RES
    _LAST_RES = res

    O = np.stack([res.results[c]["out"] for c in range(N_CORES)])
    O = O.reshape(N_CORES, C_OUT, IMG_PER_CORE, H, W_IMG)
    y = O.transpose(0, 2, 3, 4, 1).reshape(16, H, W_IMG, C_OUT)
    return np.ascontiguousarray(y).astype(np.float32)
